# revision 4
# baseline (speedup 1.0000x reference)
"""Causal self-attention (RoPE) Trainium2 kernel, 8-core SPMD.

Sharding: core c -> (batch b = c//2, head-group g = c%2). Each core computes
its batch's attention output restricted to its 8 heads, then applies the
corresponding 512 rows of Wo^T; the host sums the two head-group partials
per batch (Megatron-style row-parallel output projection).

All matmul operands are bf16 (fp32 PSUM accumulation); inputs are
host-retiled so every DMA is one contiguous 8KB run per partition.
Device layout is fully transposed (features on partitions): QT/KT = W @ X^T
with RoPE applied via a PE rotation matmul (sin-mul folded into the PSUM
read); scoresT = K_j @ Q^T per 128-row s2 block, causal-clipped exactly;
exp on ScalarE -> bf16 probs; OT_aug = [V | 1]^T @ P^T accumulates PV and
the softmax denominator in one PSUM tile; out = OT^T @ Wo as 128-row
blocks with the denominator division folded in before projection.

Schedule: a minimal prologue (K/Q for head-pair 0, V chunks 0,1) starts
the exp engine (~16.8M exps/core is the long pole) as early as possible;
the four c0 attention units run first, with the remaining projection work
interleaved as PE filler; the c1 units follow with output-projection
blocks filling their j-loops. Softmax normalization is split into
512-column halves so the final projection starts before the last unit
drains. PSUM banks: prologue acc pool 6; then acc 2 + score tiles 4 +
PV accumulator 2 (rotation outputs share the score pool).
"""

import math

import numpy as np
import ml_dtypes

BF16 = ml_dtypes.bfloat16

B, S, DIM = 4, 2048, 1024
NUM_HEADS = 16
HEAD_DIM = 64
ROPE_BASE = 10000.0
N_CORES = 8
HG = 8          # heads per core (head-group)
O = HG * HEAD_DIM  # 512 per-core projection width
C1 = 1024       # attention s1 chunk width

_NC = None  # cached compiled Bass program


def _rope_tables():
    inv_freq = 1.0 / (ROPE_BASE ** (np.arange(0, HEAD_DIM, 2, dtype=np.float64) / HEAD_DIM))
    t = np.arange(S, dtype=np.float64)
    freqs = np.einsum("i,j->ij", t, inv_freq)          # (S, 32)
    emb = np.concatenate([freqs, freqs], axis=-1)      # (S, 64)
    cos = np.cos(emb).astype(np.float32)
    sin = np.sin(emb).astype(np.float32)
    # transposed + tiled to 128 partitions (2 heads per 128-row tile)
    cosT = np.tile(cos.T, (2, 1))                      # (128, S)
    sinT = np.tile(sin.T, (2, 1))
    return cosT, sinT


def _rot_matrix():
    # rotate_half as a matrix: out[d] = -q[d+32] (d<32), q[d-32] (d>=32)
    r = np.zeros((HEAD_DIM, HEAD_DIM), dtype=np.float32)
    for d in range(32):
        r[d, d + 32] = -1.0
        r[d + 32, d] = 1.0
    r128 = np.zeros((128, 128), dtype=np.float32)
    r128[:64, :64] = r
    r128[64:, 64:] = r
    return r128.T.copy()  # lhsT for out = R @ q


def _build_nc(reps=1):
    from contextlib import ExitStack

    import concourse.tile as tile
    from concourse import bacc
    import concourse.mybir as mybir

    f32 = mybir.dt.float32
    bf16 = mybir.dt.bfloat16

    nc = bacc.Bacc("TRN2", target_bir_lowering=False, debug=False,
                   num_devices=N_CORES)

    # host-retiled layouts: one contiguous 8KB run per partition per DMA
    xT = nc.declare_dram_parameter("xTt", [4 * 128, 4096], bf16,
                                   isOutput=False)
    wqT = nc.declare_dram_parameter("wqTt", [128, 4096], bf16, isOutput=False)
    wkT = nc.declare_dram_parameter("wkTt", [128, 4096], bf16, isOutput=False)
    wvT = nc.declare_dram_parameter("wvTt", [128, 4096], bf16, isOutput=False)
    woT = nc.declare_dram_parameter("woTt", [128, 4096], bf16, isOutput=False)
    cosT = nc.declare_dram_parameter("cosT", [128, S], bf16, isOutput=False)
    sinT = nc.declare_dram_parameter("sinT", [128, S], bf16, isOutput=False)
    rT = nc.declare_dram_parameter("rT", [128, 128], bf16, isOutput=False)
    dmask = nc.declare_dram_parameter("dmask", [128, 128], bf16, isOutput=False)
    out = nc.declare_dram_parameter("out", [S, DIM], bf16, isOutput=True)

    with tile.TileContext(nc) as tc, ExitStack() as top:
        for _ in range(reps):
            _emit_body(nc, tc, mybir, xT, wqT, wkT, wvT, woT,
                       cosT, sinT, rT, dmask, out)

    nc.compile()
    return nc


def _emit_body(nc, tc, mybir, xT, wqT, wkT, wvT, woT,
               cosT, sinT, rT, dmask, out):
    from contextlib import ExitStack

    f32 = mybir.dt.float32
    bf16 = mybir.dt.bfloat16

    xT3 = xT.ap().rearrange("(c p) f -> p c f", p=128)      # (128, 4, 4096)

    with ExitStack() as top:
        otp = top.enter_context(tc.tile_pool(name="otp", bufs=1))
        qk = top.enter_context(tc.tile_pool(name="qk", bufs=1))
        cst = top.enter_context(tc.tile_pool(name="cst", bufs=1))
        xp = top.enter_context(tc.tile_pool(name="xp", bufs=4))
        wp = top.enter_context(tc.tile_pool(name="wp", bufs=3))
        wop = top.enter_context(tc.tile_pool(name="wop", bufs=1))
        tp = top.enter_context(tc.tile_pool(name="tp", bufs=4))
        ep = top.enter_context(tc.tile_pool(name="ep", bufs=4))
        rp = top.enter_context(tc.tile_pool(name="rp", bufs=2))
        bp = top.enter_context(tc.tile_pool(name="bp", bufs=2))
        stg = top.enter_context(tc.tile_pool(name="stg", bufs=3))

        OT = otp.tile([128, 4, S], bf16)   # normalized attn out, transposed
        QT = qk.tile([128, 4, S], bf16)
        KT = qk.tile([128, 4, S], bf16)
        VA = qk.tile([128, 16, 520], bf16)  # [V(64) | ones] per head

        cos_sb = cst.tile([128, S], bf16)
        sin_sb = cst.tile([128, S], bf16)
        rt_sb = cst.tile([128, 128], bf16)
        dm_sb = cst.tile([128, 128], bf16)
        wo_sb = wop.tile([128, 4096], bf16)

        # ---------------- DMA issue (order = priority) ----------------
        # every transfer is one 8KB-contiguous run per partition
        wk_sb = wp.tile([128, 4096], bf16, tag="w", name="wk_sb")
        nc.sync.dma_start(wk_sb[:, 0:2048], wkT.ap()[:, 0:2048])
        xs = []
        for sc in range(4):
            x_sb = xp.tile([128, 4096], bf16, tag="x", name=f"x{sc}")
            if sc < 2:
                nc.sync.dma_start(x_sb[:, 0:2048], xT3[:, sc, 0:2048])
                nc.sync.dma_start(x_sb[:, 2048:4096], xT3[:, sc, 2048:4096])
            xs.append(x_sb)
        nc.sync.dma_start(wk_sb[:, 2048:4096], wkT.ap()[:, 2048:4096])
        wv_sb = wp.tile([128, 4096], bf16, tag="w", name="wv_sb")
        nc.sync.dma_start(wv_sb[:], wvT.ap())
        wq_sb = wp.tile([128, 4096], bf16, tag="w", name="wq_sb")
        nc.sync.dma_start(wq_sb[:], wqT.ap())
        for sc in range(2, 4):
            nc.sync.dma_start(xs[sc][:], xT3[:, sc, :])
        nc.gpsimd.dma_start(cos_sb[:], cosT.ap())
        nc.gpsimd.dma_start(sin_sb[:], sinT.ap())
        nc.gpsimd.dma_start(rt_sb[:], rT.ap())
        nc.gpsimd.dma_start(dm_sb[:], dmask.ap())

        wkt = wk_sb[:].rearrange("p (o f) -> p o f", f=512)   # (128, 8, 512)
        wvt = wv_sb[:].rearrange("p (o f) -> p o f", f=512)
        wqt = wq_sb[:].rearrange("p (o f) -> p o f", f=512)

        def xtile(sc, kt, csl):
            xv = xs[sc][:].rearrange("p (o s) -> p o s", s=512)
            return xv[:, kt, csl]

        P = {}  # current-scope PSUM pools: "pp", "pss", "pso"

        def rope(acc, dest, ot, sl):
            # dest[:, ot, sl] = acc*cos + R @ (acc*sin)
            rs = tp.tile([128, 512], bf16, tag="t", name="rs")
            nc.vector.tensor_mul(rs[:], acc[:], sin_sb[:, sl])
            rot = P["rotp"].tile([128, 512], f32, tag=P["rottag"],
                                 name="rot")
            nc.tensor.matmul(rot[:], rt_sb[:], rs[:], start=True, stop=True)
            t1 = tp.tile([128, 512], f32, tag="t", name="t1")
            nc.vector.tensor_mul(t1[:], acc[:], cos_sb[:, sl])
            nc.vector.tensor_add(dest[:, ot, sl], t1[:], rot[:])

        def emit_kq(wlist, dest, ot, sc):
            sl = slice(sc * 512, (sc + 1) * 512)
            acc = P["pp"].tile([128, 512], f32, tag="pp", name="acc")
            for kt in range(8):
                nc.tensor.matmul(
                    acc[:],
                    wlist[:, kt, ot * 128:(ot + 1) * 128],
                    xtile(sc, kt, slice(0, 512)),
                    start=(kt == 0), stop=(kt == 7))
            rope(acc, dest, ot, sl)

        def emit_v(sc):
            for st in range(4):
                s2t = sc * 4 + st
                acc = P["pp"].tile([128, 512], f32, tag="pp", name="acc")
                for kt in range(8):
                    nc.tensor.matmul(
                        acc[:],
                        xtile(sc, kt, slice(st * 128, (st + 1) * 128)),
                        wvt[:, kt, :],
                        start=(kt == 0), stop=(kt == 7))
                vsl = VA[:, s2t, :].rearrange("p (h c) -> p h c", c=65)
                nc.scalar.copy(
                    vsl[:, :, 0:64],
                    acc[:].rearrange("p (h c) -> p h c", c=64))
                nc.gpsimd.memset(vsl[:, :, 64:65], 1.0)

        def emit_attn(c, ot, hb, fill=None, fill_start=0):
            # one head: s1 chunk c (1024 wide), head 2*ot + (hb!=0)
            # fill: list of zero-arg callables emitted one-per-j as PE gap
            # filler work (pool-slot FIFO makes emission order ~execution
            # order, so filler must be interleaved at this granularity)
            h = 2 * ot + (1 if hb else 0)
            otps = P["pso"].tile([65, C1], f32, tag="otps", name="otps")

            def emit_pv(j, l0, et):
                for n in range(2):
                    if 512 * (n + 1) <= l0:
                        continue
                    ln = max(l0, 512 * n)
                    nc.tensor.matmul(
                        otps[:, ln:512 * (n + 1)],
                        VA[:, j, h * 65:(h + 1) * 65],
                        et[:, ln:512 * (n + 1)],
                        start=(j == 0), stop=(j == 8 * c + 4 * n + 3))

            def norm_half(n):
                # denominator row 64 of otps; cols [512n, 512n+512)
                nsl = slice(512 * n, 512 * (n + 1))
                rec = rp.tile([1, 512], f32, tag="rec", name="rec")
                nc.vector.reciprocal(rec[:], otps[64:65, nsl])
                bc = bp.tile([64, 512], f32, tag="bc", name="bc")
                nc.gpsimd.partition_broadcast(bc[:], rec[:])
                nc.vector.tensor_mul(
                    OT[hb:hb + 64, ot, c * C1 + 512 * n:c * C1 + 512 * (n + 1)],
                    otps[0:64, nsl], bc[:])

            pending = None
            for j in range(8 * c + 8):      # s2 blocks of 128
                l0 = max(0, 128 * j - C1 * c)
                sc_ps = P["pss"].tile([128, C1], f32, tag="sc", name="sc_ps")
                for n in range(2):
                    base = 512 * n
                    if base + 512 <= l0:
                        continue
                    lo = max(l0, base)
                    nc.tensor.matmul(
                        sc_ps[:, lo:base + 512],
                        KT[hb:hb + 64, ot, j * 128:(j + 1) * 128],
                        QT[hb:hb + 64, ot, c * C1 + lo:c * C1 + base + 512],
                        start=True, stop=True)
                et = ep.tile([128, C1], bf16, tag="e", name="et")
                nc.scalar.activation(
                    et[:, l0:C1], sc_ps[:, l0:C1],
                    mybir.ActivationFunctionType.Exp,
                    scale=1.0 / math.sqrt(HEAD_DIM))
                if 128 * j >= C1 * c:
                    dl = 128 * j - C1 * c
                    nc.gpsimd.tensor_mul(
                        et[:, dl:dl + 128], et[:, dl:dl + 128], dm_sb[:])
                if pending is not None:
                    emit_pv(*pending)
                    if pending[0] == 8 * c + 3:  # last n=0 contribution done
                        norm_half(0)
                if fill and j >= fill_start:
                    fill.pop(0)()
                pending = (j, l0, et)
            emit_pv(*pending)
            norm_half(1)
            while fill:
                fill.pop(0)()

        def proj_block(sb):
            st = stg.tile([128, DIM], bf16, tag="st", name="st")
            pj = P["pss"].tile([128, C1], f32, tag="sc", name="pj")
            wov = wo_sb[:].rearrange("p (o f) -> p o f", f=1024)
            for half in range(2):
                for kt in range(4):
                    nc.tensor.matmul(
                        pj[:, half * 512:(half + 1) * 512],
                        OT[:, kt, sb * 128:(sb + 1) * 128],
                        wov[:, kt, half * 512:(half + 1) * 512],
                        start=(kt == 0), stop=(kt == 3))
                nc.vector.tensor_copy(
                    st[:, half * 512:(half + 1) * 512],
                    pj[:, half * 512:(half + 1) * 512])
            nc.sync.dma_start(
                out.ap()[sb * 128:(sb + 1) * 128, :], st[:])

        # ---------------- emission schedule ----------------
        # Scope 1: minimal pre-attention prologue (K chunks 0,1; V chunk 0;
        # Q(ot0) cols 0:1024) at full acc-pipeline depth.
        with ExitStack() as s1:
            P["pp"] = s1.enter_context(
                tc.tile_pool(name="pp1", bufs=6, space="PSUM"))
            P["rotp"] = P["pp"]
            P["rottag"] = "pp"
            for sc in (0, 1):
                for ot in range(4):
                    emit_kq(wkt, KT, ot, sc)
            emit_v(0)
            emit_kq(wqt, QT, 0, 0)
            emit_kq(wqt, QT, 0, 1)

        # Scope 2: uniform pipeline, c0 attention units first (they only
        # need chunks 0,1), with the rest of phase 1 interleaved as PE
        # filler sized to the exp engine's idle windows; then the four c1
        # units with the remaining Q chunks and projection blocks filling.
        with ExitStack() as s2:
            P["pp"] = s2.enter_context(
                tc.tile_pool(name="pp2", bufs=2, space="PSUM"))
            P["pss"] = s2.enter_context(
                tc.tile_pool(name="pss", bufs=2, space="PSUM"))
            P["pso"] = s2.enter_context(
                tc.tile_pool(name="pso", bufs=1, space="PSUM"))
            P["rotp"] = P["pss"]
            P["rottag"] = "sc"
            emit_v(1)
            emit_attn(0, 0, 0)
            emit_attn(0, 0, 64)
            emit_kq(wqt, QT, 1, 0)
            emit_kq(wqt, QT, 1, 1)
            emit_attn(0, 1, 0)
            emit_attn(0, 1, 64)
            emit_kq(wqt, QT, 2, 0)
            emit_kq(wqt, QT, 2, 1)
            for ot in range(4):
                emit_kq(wkt, KT, ot, 2)
            emit_v(2)
            emit_attn(0, 2, 0)
            emit_attn(0, 2, 64)
            emit_kq(wqt, QT, 3, 0)
            emit_kq(wqt, QT, 3, 1)
            for ot in range(4):
                emit_kq(wkt, KT, ot, 3)
            emit_v(3)
            emit_attn(0, 3, 0)
            emit_attn(0, 3, 64)
            nc.gpsimd.dma_start(wo_sb[:], woT.ap())
            emit_kq(wqt, QT, 0, 2)
            emit_kq(wqt, QT, 0, 3)
            emit_attn(1, 0, 0)
            emit_attn(1, 0, 64)
            emit_kq(wqt, QT, 1, 2)
            emit_kq(wqt, QT, 1, 3)
            emit_attn(1, 1, 0)
            emit_attn(1, 1, 64)
            emit_kq(wqt, QT, 2, 2)
            emit_kq(wqt, QT, 2, 3)
            emit_attn(1, 2, 0)
            emit_attn(1, 2, 64)
            emit_kq(wqt, QT, 3, 2)
            emit_kq(wqt, QT, 3, 3)
            emit_attn(1, 3, 0,
                      fill=[lambda sb=sb: proj_block(sb) for sb in range(8)])
            # proj blocks 8-11 read this unit's own first half-norm, so
            # they may only be emitted from j=12 on; 12-15 drain after the
            # final half-norm
            emit_attn(1, 3, 64,
                      fill=[lambda sb=sb: proj_block(sb)
                            for sb in range(8, 16)],
                      fill_start=12)


def _get_nc():
    global _NC
    if _NC is None:
        _NC = _build_nc()
    return _NC


def _retile_w(wt, o):
    # (o*128, f) -> (128, o*f): per-partition contiguous k-chunk-major
    f = wt.shape[1]
    return np.ascontiguousarray(
        wt.reshape(o, 128, f).transpose(1, 0, 2).reshape(128, o * f))


def make_in_maps(x, Wq, Wk, Wv, Wo):
    cosT, sinT = _rope_tables()
    rT = _rot_matrix().astype(BF16)
    # keep where s2 <= s1 in (s2, s1) indexing -> upper-tri incl diag
    dm = np.triu(np.ones((128, 128), dtype=BF16))
    in_maps = []
    for c in range(N_CORES):
        b, g = c // 2, c % 2
        rows = slice(g * O, (g + 1) * O)
        xt = x[b].T.astype(BF16).reshape(8, 128, S)
        xtt = np.stack([
            np.ascontiguousarray(
                xt[:, :, sc * 512:(sc + 1) * 512]
            ).transpose(1, 0, 2).reshape(128, 4096)
            for sc in range(4)], axis=0).reshape(512, 4096)
        in_maps.append({
            "xTt": np.ascontiguousarray(xtt),
            "wqTt": _retile_w(Wq[rows, :].T.astype(BF16), 8),
            "wkTt": _retile_w(Wk[rows, :].T.astype(BF16), 8),
            "wvTt": _retile_w(Wv[rows, :].T.astype(BF16), 8),
            "woTt": _retile_w(Wo[:, rows].T.astype(BF16), 4),
            "cosT": cosT.astype(BF16), "sinT": sinT.astype(BF16),
            "rT": rT, "dmask": dm,
        })
    return in_maps


def _numpy_fallback(x, Wq, Wk, Wv, Wo, mask):
    cosT, sinT = _rope_tables()
    cos, sin = cosT[:64].T, sinT[:64].T                      # (S, 64)
    xq = x @ Wq.T
    xk = x @ Wk.T
    xv = x @ Wv.T

    def heads(t):
        return t.reshape(B, S, NUM_HEADS, HEAD_DIM).transpose(0, 2, 1, 3)

    q, k, v = heads(xq), heads(xk), heads(xv)

    def rot(t):
        return np.concatenate([-t[..., 32:], t[..., :32]], axis=-1)

    q = q * cos + rot(q) * sin
    k = k * cos + rot(k) * sin
    sc = np.einsum("bhsd,bhtd->bhst", q, k) / math.sqrt(HEAD_DIM)
    sc = np.where(mask[None, None] == 0, -np.inf, sc)
    sc = sc - sc.max(axis=-1, keepdims=True)
    e = np.exp(sc)
    p = e / e.sum(axis=-1, keepdims=True)
    o = np.einsum("bhst,bhtd->bhsd", p, v)
    o = o.transpose(0, 2, 1, 3).reshape(B, S, DIM)
    return (o @ Wo.T).astype(np.float32)


def kernel(x, Wq, Wk, Wv, Wo, mask):
    x = np.asarray(x)
    mask = np.asarray(mask)
    causal = bool(
        np.array_equal(np.asarray(mask, dtype=np.int64),
                       np.tril(np.ones((S, S), dtype=np.int64))))
    if not causal:
        return _numpy_fallback(
            np.asarray(x, np.float32), np.asarray(Wq, np.float32),
            np.asarray(Wk, np.float32), np.asarray(Wv, np.float32),
            np.asarray(Wo, np.float32), mask)

    from concourse.bass_utils import run_bass_kernel_spmd

    nc = _get_nc()
    in_maps = make_in_maps(x, Wq, Wk, Wv, Wo)
    res = run_bass_kernel_spmd(nc, in_maps, list(range(N_CORES)))
    out = np.empty((B, S, DIM), dtype=np.float32)
    for b in range(B):
        out[b] = (res.results[2 * b]["out"].astype(np.float32)
                  + res.results[2 * b + 1]["out"].astype(np.float32))
    return out


# revision 5
# speedup vs baseline: 5.2190x; 5.2190x over previous
"""Causal self-attention (RoPE) Trainium2 kernel, 8-core SPMD. v3.

Sharding: core c -> (batch b = c//2, head-group g = c%2). Each core computes
its batch's attention output restricted to its 8 heads, then applies the
corresponding 512 rows of Wo^T; the host sums the two head-group partials
per batch (Megatron-style row-parallel output projection).

Single fully-pipelined schedule: projections (bf16, fp32 PSUM), RoPE via
PE rotation matmul with the sin-mul folded into the PSUM read, attention
emitted ot-major and hand-interleaved with the remaining projection chunks
so the exp engine (the long pole, ~16.8M exps/core) starts at ~30us and
stays busy under PE-bound work. All eight PSUM banks are statically
partitioned: phase1 acc/rot 2, score tiles 4, PV accumulators 2.
"""

import math

import numpy as np
import ml_dtypes

BF16 = ml_dtypes.bfloat16

B, S, DIM = 4, 2048, 1024
NUM_HEADS = 16
HEAD_DIM = 64
ROPE_BASE = 10000.0
N_CORES = 8
HG = 8          # heads per core (head-group)
O = HG * HEAD_DIM  # 512 per-core projection width
C1 = 1024       # attention s1 chunk width

_NC = None  # cached compiled Bass program


def _rope_tables():
    inv_freq = 1.0 / (ROPE_BASE ** (np.arange(0, HEAD_DIM, 2, dtype=np.float64) / HEAD_DIM))
    t = np.arange(S, dtype=np.float64)
    freqs = np.einsum("i,j->ij", t, inv_freq)          # (S, 32)
    emb = np.concatenate([freqs, freqs], axis=-1)      # (S, 64)
    cos = np.cos(emb).astype(np.float32)
    sin = np.sin(emb).astype(np.float32)
    # transposed + tiled to 128 partitions (2 heads per 128-row tile)
    cosT = np.tile(cos.T, (2, 1))                      # (128, S)
    sinT = np.tile(sin.T, (2, 1))
    return cosT, sinT


def _rot_matrix():
    # rotate_half as a matrix: out[d] = -q[d+32] (d<32), q[d-32] (d>=32)
    r = np.zeros((HEAD_DIM, HEAD_DIM), dtype=np.float32)
    for d in range(32):
        r[d, d + 32] = -1.0
        r[d + 32, d] = 1.0
    r128 = np.zeros((128, 128), dtype=np.float32)
    r128[:64, :64] = r
    r128[64:, 64:] = r
    return r128.T.copy()  # lhsT for out = R @ q


def _build_nc(reps=1):
    from contextlib import ExitStack

    import concourse.tile as tile
    from concourse import bacc
    import concourse.mybir as mybir

    f32 = mybir.dt.float32
    bf16 = mybir.dt.bfloat16

    nc = bacc.Bacc("TRN2", target_bir_lowering=False, debug=False,
                   num_devices=N_CORES)

    # host-retiled layouts: one contiguous 8KB run per partition per DMA
    xT = nc.declare_dram_parameter("xTt", [4 * 128, 4096], bf16,
                                   isOutput=False)
    wqT = nc.declare_dram_parameter("wqTt", [128, 4096], bf16, isOutput=False)
    wkT = nc.declare_dram_parameter("wkTt", [128, 4096], bf16, isOutput=False)
    wvT = nc.declare_dram_parameter("wvTt", [128, 4096], bf16, isOutput=False)
    woT = nc.declare_dram_parameter("woTt", [128, 4096], bf16, isOutput=False)
    cosT = nc.declare_dram_parameter("cosT", [128, S], bf16, isOutput=False)
    sinT = nc.declare_dram_parameter("sinT", [128, S], bf16, isOutput=False)
    rT = nc.declare_dram_parameter("rT", [128, 128], bf16, isOutput=False)
    dmask = nc.declare_dram_parameter("dmask", [128, 128], bf16, isOutput=False)
    out = nc.declare_dram_parameter("out", [S, DIM], bf16, isOutput=True)

    with tile.TileContext(nc) as tc, ExitStack() as top:
        for _ in range(reps):
            _emit_body(nc, tc, mybir, xT, wqT, wkT, wvT, woT,
                       cosT, sinT, rT, dmask, out)

    nc.compile()
    return nc


def _emit_body(nc, tc, mybir, xT, wqT, wkT, wvT, woT,
               cosT, sinT, rT, dmask, out):
    from contextlib import ExitStack

    f32 = mybir.dt.float32
    bf16 = mybir.dt.bfloat16

    xT3 = xT.ap().rearrange("(c p) f -> p c f", p=128)      # (128, 4, 4096)

    with ExitStack() as top:
        otp = top.enter_context(tc.tile_pool(name="otp", bufs=1))
        qk = top.enter_context(tc.tile_pool(name="qk", bufs=1))
        cst = top.enter_context(tc.tile_pool(name="cst", bufs=1))
        xp = top.enter_context(tc.tile_pool(name="xp", bufs=4))
        wp = top.enter_context(tc.tile_pool(name="wp", bufs=3))
        wop = top.enter_context(tc.tile_pool(name="wop", bufs=1))
        tp = top.enter_context(tc.tile_pool(name="tp", bufs=4))
        ep = top.enter_context(tc.tile_pool(name="ep", bufs=6))
        rp = top.enter_context(tc.tile_pool(name="rp", bufs=2))
        bp = top.enter_context(tc.tile_pool(name="bp", bufs=2))
        stg = top.enter_context(tc.tile_pool(name="stg", bufs=4))

        OT = otp.tile([128, 4, S], bf16)   # normalized attn out, transposed
        QT = qk.tile([128, 4, S], bf16)
        KT = qk.tile([128, 4, S], bf16)
        VA = qk.tile([128, 16, 520], bf16)  # [V(64) | ones] per head

        cos_sb = cst.tile([128, S], bf16)
        sin_sb = cst.tile([128, S], bf16)
        rt_sb = cst.tile([128, 128], bf16)
        dm_sb = cst.tile([128, 128], bf16)
        wo_sb = wop.tile([128, 4096], bf16)

        # ---------------- DMA issue (order = priority) ----------------
        # every transfer is one 8KB-contiguous run per partition
        wk_sb = wp.tile([128, 4096], bf16, tag="w", name="wk_sb")
        nc.sync.dma_start(wk_sb[:, 0:2048], wkT.ap()[:, 0:2048])
        xs = []
        for sc in range(4):
            x_sb = xp.tile([128, 4096], bf16, tag="x", name=f"x{sc}")
            if sc < 2:
                nc.sync.dma_start(x_sb[:, 0:2048], xT3[:, sc, 0:2048])
                nc.sync.dma_start(x_sb[:, 2048:4096], xT3[:, sc, 2048:4096])
            xs.append(x_sb)
        nc.sync.dma_start(wk_sb[:, 2048:4096], wkT.ap()[:, 2048:4096])
        wv_sb = wp.tile([128, 4096], bf16, tag="w", name="wv_sb")
        nc.sync.dma_start(wv_sb[:], wvT.ap())
        wq_sb = wp.tile([128, 4096], bf16, tag="w", name="wq_sb")
        nc.sync.dma_start(wq_sb[:], wqT.ap())
        for sc in range(2, 4):
            nc.sync.dma_start(xs[sc][:], xT3[:, sc, :])
        nc.gpsimd.dma_start(cos_sb[:], cosT.ap())
        nc.gpsimd.dma_start(sin_sb[:], sinT.ap())
        nc.gpsimd.dma_start(rt_sb[:], rT.ap())
        nc.gpsimd.dma_start(dm_sb[:], dmask.ap())

        wkt = wk_sb[:].rearrange("p (o f) -> p o f", f=512)   # (128, 8, 512)
        wvt = wv_sb[:].rearrange("p (o f) -> p o f", f=512)
        wqt = wq_sb[:].rearrange("p (o f) -> p o f", f=512)

        def xtile(sc, kt, csl):
            xv = xs[sc][:].rearrange("p (o s) -> p o s", s=512)
            return xv[:, kt, csl]

        P = {}  # current-scope PSUM pools: "pp", "pss", "pso"

        def rope(acc, dest, ot, sl):
            # dest[:, ot, sl] = acc*cos + R @ (acc*sin)
            rs = tp.tile([128, 512], bf16, tag="t", name="rs")
            nc.vector.tensor_mul(rs[:], acc[:], sin_sb[:, sl])
            rot = P["rotp"].tile([128, 512], f32, tag=P["rottag"],
                                 name="rot")
            nc.tensor.matmul(rot[:], rt_sb[:], rs[:], start=True, stop=True)
            t1 = tp.tile([128, 512], f32, tag="t", name="t1")
            nc.vector.tensor_mul(t1[:], acc[:], cos_sb[:, sl])
            nc.vector.tensor_add(dest[:, ot, sl], t1[:], rot[:])

        def emit_kq(wlist, dest, ot, sc):
            sl = slice(sc * 512, (sc + 1) * 512)
            acc = P["pp"].tile([128, 512], f32, tag="pp", name="acc")
            for kt in range(8):
                nc.tensor.matmul(
                    acc[:],
                    wlist[:, kt, ot * 128:(ot + 1) * 128],
                    xtile(sc, kt, slice(0, 512)),
                    start=(kt == 0), stop=(kt == 7))
            rope(acc, dest, ot, sl)

        def emit_v(sc):
            for st in range(4):
                s2t = sc * 4 + st
                acc = P["pp"].tile([128, 512], f32, tag="pp", name="acc")
                for kt in range(8):
                    nc.tensor.matmul(
                        acc[:],
                        xtile(sc, kt, slice(st * 128, (st + 1) * 128)),
                        wvt[:, kt, :],
                        start=(kt == 0), stop=(kt == 7))
                vsl = VA[:, s2t, :].rearrange("p (h c) -> p h c", c=65)
                nc.scalar.copy(
                    vsl[:, :, 0:64],
                    acc[:].rearrange("p (h c) -> p h c", c=64))
                nc.gpsimd.memset(vsl[:, :, 64:65], 1.0)

        def emit_attn(c, ot, hb, fill=None, fill_start=0):
            # one head: s1 chunk c (1024 wide), head 2*ot + (hb!=0)
            # fill: list of zero-arg callables emitted one-per-j as PE gap
            # filler work (pool-slot FIFO makes emission order ~execution
            # order, so filler must be interleaved at this granularity)
            h = 2 * ot + (1 if hb else 0)
            otps = P["pso"].tile([65, C1], f32, tag="otps", name="otps")

            def emit_pv(j, l0, et):
                for n in range(2):
                    if 512 * (n + 1) <= l0:
                        continue
                    ln = max(l0, 512 * n)
                    nc.tensor.matmul(
                        otps[:, ln:512 * (n + 1)],
                        VA[:, j, h * 65:(h + 1) * 65],
                        et[:, ln:512 * (n + 1)],
                        start=(j == 0), stop=(j == 8 * c + 4 * n + 3))

            def norm_half(n):
                # denominator row 64 of otps; cols [512n, 512n+512)
                nsl = slice(512 * n, 512 * (n + 1))
                rec = rp.tile([1, 512], f32, tag="rec", name="rec")
                nc.vector.reciprocal(rec[:], otps[64:65, nsl])
                bc = bp.tile([64, 512], f32, tag="bc", name="bc")
                nc.gpsimd.partition_broadcast(bc[:], rec[:])
                nc.vector.tensor_mul(
                    OT[hb:hb + 64, ot, c * C1 + 512 * n:c * C1 + 512 * (n + 1)],
                    otps[0:64, nsl], bc[:])

            pending = None
            for j in range(8 * c + 8):      # s2 blocks of 128
                l0 = max(0, 128 * j - C1 * c)
                sc_ps = P["pss"].tile([128, C1], f32, tag="sc", name="sc_ps")
                for n in range(2):
                    base = 512 * n
                    if base + 512 <= l0:
                        continue
                    lo = max(l0, base)
                    nc.tensor.matmul(
                        sc_ps[:, lo:base + 512],
                        KT[hb:hb + 64, ot, j * 128:(j + 1) * 128],
                        QT[hb:hb + 64, ot, c * C1 + lo:c * C1 + base + 512],
                        start=True, stop=True)
                et = ep.tile([128, C1], bf16, tag="e", name="et")
                nc.scalar.activation(
                    et[:, l0:C1], sc_ps[:, l0:C1],
                    mybir.ActivationFunctionType.Exp,
                    scale=1.0 / math.sqrt(HEAD_DIM))
                if 128 * j >= C1 * c:
                    dl = 128 * j - C1 * c
                    nc.gpsimd.tensor_mul(
                        et[:, dl:dl + 128], et[:, dl:dl + 128], dm_sb[:])
                if pending is not None:
                    emit_pv(*pending)
                    if pending[0] == 8 * c + 3:  # last n=0 contribution done
                        norm_half(0)
                if fill and j >= fill_start:
                    fill.pop(0)()
                pending = (j, l0, et)
            emit_pv(*pending)
            norm_half(1)
            while fill:
                fill.pop(0)()

        def proj_block(sb):
            st = stg.tile([128, DIM], bf16, tag="st", name="st")
            pj = P["pss"].tile([128, C1], f32, tag="sc", name="pj")
            wov = wo_sb[:].rearrange("p (o f) -> p o f", f=1024)
            for half in range(2):
                for kt in range(4):
                    nc.tensor.matmul(
                        pj[:, half * 512:(half + 1) * 512],
                        OT[:, kt, sb * 128:(sb + 1) * 128],
                        wov[:, kt, half * 512:(half + 1) * 512],
                        start=(kt == 0), stop=(kt == 3))
                nc.vector.tensor_copy(
                    st[:, half * 512:(half + 1) * 512],
                    pj[:, half * 512:(half + 1) * 512])
            nc.sync.dma_start(
                out.ap()[sb * 128:(sb + 1) * 128, :], st[:])

        # ---------------- emission schedule ----------------
        # Scope 1: minimal pre-attention prologue -- only what attention
        # unit (ot0, c0) actually reads: K(ot0) and Q(ot0) over cols
        # 0:1024 plus V chunks 0,1 -- at full acc-pipeline depth.
        with ExitStack() as s1:
            P["pp"] = s1.enter_context(
                tc.tile_pool(name="pp1", bufs=6, space="PSUM"))
            P["rotp"] = P["pp"]
            P["rottag"] = "pp"
            emit_kq(wkt, KT, 0, 0)
            emit_kq(wkt, KT, 0, 1)
            emit_kq(wqt, QT, 0, 0)
            emit_kq(wqt, QT, 0, 1)
            emit_v(0)
            emit_v(1)

        # Scope 2: uniform pipeline, c0 attention units first (they only
        # need chunks 0,1), with the rest of phase 1 interleaved as PE
        # filler sized to the exp engine's idle windows; then the four c1
        # units with the remaining Q chunks and projection blocks filling.
        with ExitStack() as s2:
            P["pp"] = s2.enter_context(
                tc.tile_pool(name="pp2", bufs=2, space="PSUM"))
            P["pss"] = s2.enter_context(
                tc.tile_pool(name="pss", bufs=2, space="PSUM"))
            P["pso"] = s2.enter_context(
                tc.tile_pool(name="pso", bufs=1, space="PSUM"))
            P["rotp"] = P["pss"]
            P["rottag"] = "sc"
            emit_attn(0, 0, 0)
            emit_attn(0, 0, 64)
            for ot in (1, 2, 3):
                emit_kq(wkt, KT, ot, 0)
                emit_kq(wkt, KT, ot, 1)
            emit_kq(wqt, QT, 1, 0)
            emit_kq(wqt, QT, 1, 1)
            emit_attn(0, 1, 0)
            emit_attn(0, 1, 64)
            emit_kq(wqt, QT, 2, 0)
            emit_kq(wqt, QT, 2, 1)
            for ot in range(4):
                emit_kq(wkt, KT, ot, 2)
            emit_v(2)
            emit_attn(0, 2, 0)
            emit_attn(0, 2, 64)
            emit_kq(wqt, QT, 3, 0)
            emit_kq(wqt, QT, 3, 1)
            for ot in range(4):
                emit_kq(wkt, KT, ot, 3)
            emit_v(3)
            emit_attn(0, 3, 0)
            emit_attn(0, 3, 64)
            nc.gpsimd.dma_start(wo_sb[:], woT.ap())
            emit_kq(wqt, QT, 0, 2)
            emit_kq(wqt, QT, 0, 3)
            emit_attn(1, 0, 0)
            emit_attn(1, 0, 64,
                      fill=[lambda sb=sb: proj_block(sb) for sb in range(3)])
            emit_kq(wqt, QT, 1, 2)
            emit_kq(wqt, QT, 1, 3)
            emit_attn(1, 1, 0)
            emit_attn(1, 1, 64,
                      fill=[lambda sb=sb: proj_block(sb) for sb in range(3, 6)])
            emit_kq(wqt, QT, 2, 2)
            emit_kq(wqt, QT, 2, 3)
            emit_attn(1, 2, 0)
            emit_attn(1, 2, 64,
                      fill=[lambda sb=sb: proj_block(sb) for sb in range(6, 8)])
            emit_kq(wqt, QT, 3, 2)
            emit_kq(wqt, QT, 3, 3)
            emit_attn(1, 3, 0)
            # proj blocks 8-11 read this unit's own first half-norm, so
            # they may only be emitted from j=12 on; 12-15 drain after the
            # final half-norm
            emit_attn(1, 3, 64,
                      fill=[lambda sb=sb: proj_block(sb)
                            for sb in range(8, 16)],
                      fill_start=12)


def _get_nc():
    global _NC
    if _NC is None:
        _NC = _build_nc()
    return _NC


def _retile_w(wt, o):
    # (o*128, f) -> (128, o*f): per-partition contiguous k-chunk-major
    f = wt.shape[1]
    return np.ascontiguousarray(
        wt.reshape(o, 128, f).transpose(1, 0, 2).reshape(128, o * f))


def make_in_maps(x, Wq, Wk, Wv, Wo):
    cosT, sinT = _rope_tables()
    rT = _rot_matrix().astype(BF16)
    # keep where s2 <= s1 in (s2, s1) indexing -> upper-tri incl diag
    dm = np.triu(np.ones((128, 128), dtype=BF16))
    in_maps = []
    for c in range(N_CORES):
        b, g = c // 2, c % 2
        rows = slice(g * O, (g + 1) * O)
        xt = x[b].T.astype(BF16).reshape(8, 128, S)
        xtt = np.stack([
            np.ascontiguousarray(
                xt[:, :, sc * 512:(sc + 1) * 512]
            ).transpose(1, 0, 2).reshape(128, 4096)
            for sc in range(4)], axis=0).reshape(512, 4096)
        in_maps.append({
            "xTt": np.ascontiguousarray(xtt),
            "wqTt": _retile_w(Wq[rows, :].T.astype(BF16), 8),
            "wkTt": _retile_w(Wk[rows, :].T.astype(BF16), 8),
            "wvTt": _retile_w(Wv[rows, :].T.astype(BF16), 8),
            "woTt": _retile_w(Wo[:, rows].T.astype(BF16), 4),
            "cosT": cosT.astype(BF16), "sinT": sinT.astype(BF16),
            "rT": rT, "dmask": dm,
        })
    return in_maps


def _numpy_fallback(x, Wq, Wk, Wv, Wo, mask):
    cosT, sinT = _rope_tables()
    cos, sin = cosT[:64].T, sinT[:64].T                      # (S, 64)
    xq = x @ Wq.T
    xk = x @ Wk.T
    xv = x @ Wv.T

    def heads(t):
        return t.reshape(B, S, NUM_HEADS, HEAD_DIM).transpose(0, 2, 1, 3)

    q, k, v = heads(xq), heads(xk), heads(xv)

    def rot(t):
        return np.concatenate([-t[..., 32:], t[..., :32]], axis=-1)

    q = q * cos + rot(q) * sin
    k = k * cos + rot(k) * sin
    sc = np.einsum("bhsd,bhtd->bhst", q, k) / math.sqrt(HEAD_DIM)
    sc = np.where(mask[None, None] == 0, -np.inf, sc)
    sc = sc - sc.max(axis=-1, keepdims=True)
    e = np.exp(sc)
    p = e / e.sum(axis=-1, keepdims=True)
    o = np.einsum("bhst,bhtd->bhsd", p, v)
    o = o.transpose(0, 2, 1, 3).reshape(B, S, DIM)
    return (o @ Wo.T).astype(np.float32)


def kernel(x, Wq, Wk, Wv, Wo, mask):
    x = np.asarray(x)
    mask = np.asarray(mask)
    causal = bool(
        np.array_equal(np.asarray(mask, dtype=np.int64),
                       np.tril(np.ones((S, S), dtype=np.int64))))
    if not causal:
        return _numpy_fallback(
            np.asarray(x, np.float32), np.asarray(Wq, np.float32),
            np.asarray(Wk, np.float32), np.asarray(Wv, np.float32),
            np.asarray(Wo, np.float32), mask)

    from concourse.bass_utils import run_bass_kernel_spmd

    nc = _get_nc()
    in_maps = make_in_maps(x, Wq, Wk, Wv, Wo)
    res = run_bass_kernel_spmd(nc, in_maps, list(range(N_CORES)))
    out = np.empty((B, S, DIM), dtype=np.float32)
    for b in range(B):
        out[b] = (res.results[2 * b]["out"].astype(np.float32)
                  + res.results[2 * b + 1]["out"].astype(np.float32))
    return out
